# revision 25
# baseline (speedup 1.0000x reference)
"""Trainium2 Bass kernel for nn_DetNet (unfolded DetNet forward).

Strategy:
  - Pure data parallel: batch 256 -> 8 cores x 32.
  - Per core: 4 blocks of 8 batch elements. Per block, Psi_e/Psi_o are
    loaded once into SBUF (bf16, both natural [k-part, j-free] and
    transposed [j-part, m-free] layouts) and stay resident for all 10
    layers. Per-layer dense weights (bf16, bias rows packed) stream per
    (block, layer).
  - Batched per-element matvecs run on the PE by streaming Psi tiles as
    the moving operand against zero-masked per-element stationary
    columns, so all 8 rows accumulate into one [8, N] PSUM tile.
  - Everything quantized to bf16 on the host (fp32 accumulation on-chip);
    measured end-to-end rel err vs fp32 reference ~6e-4.
"""

import os
import sys

sys.path.insert(0, "/opt/trn_rl_repo")

import numpy as np
from ml_dtypes import bfloat16

import concourse.bass as bass
import concourse.bacc as bacc
import concourse.mybir as mybir
from concourse import tile
from concourse.ap import AP as _AP
from concourse.bass_utils import run_bass_kernel_spmd


def _strided(tileap, pairs):
    """View a 2D [128, F] tile AP with custom free-dim [step, count] pairs."""
    return _AP(tileap.tensor, tileap.offset,
               [list(tileap.ap[0])] + [list(p) for p in pairs])

F32 = mybir.dt.float32
BF16 = mybir.dt.bfloat16
AF = mybir.ActivationFunctionType
ALU = mybir.AluOpType
AX = mybir.AxisListType

LAYERS = int(os.environ.get("DETNET_LAYERS", "10"))
BL = 256          # BLOCK_LEN
SL = 264          # SYM_LEN
JD = 2 * SL       # 528, xvec dim
KD = 2 * BL       # 512, Psi row dim
VL = 128          # V_LEN
ZL = 512          # Z_LEN
OHM, OHP = 4, 8
BATCH = 256
NCORES = 8
BCORE = BATCH // NCORES            # 32
BB = int(os.environ.get("DETNET_BB", "8"))     # block batch
NBLK = BCORE // BB                 # 4
W1K = SL + VL + 1                  # 393 rows incl bias
PI2 = float(np.pi / 2.0)


def _ceil_div(a, b):
    return (a + b - 1) // b


def emit_core_program(nc, tc, io):
    """Emit the full per-core Tile program. io: dict name -> DRAM AP."""
    from contextlib import ExitStack

    ctx = ExitStack()
    consts = ctx.enter_context(tc.tile_pool(name="consts", bufs=1))
    psi = ctx.enter_context(tc.tile_pool(name="psi", bufs=1))
    wpool = ctx.enter_context(tc.tile_pool(name="weights", bufs=2))
    work = ctx.enter_context(tc.tile_pool(name="work", bufs=2))
    work1 = ctx.enter_context(tc.tile_pool(name="work1", bufs=1))
    carry = ctx.enter_context(tc.tile_pool(name="carry", bufs=1))
    stat = ctx.enter_context(tc.tile_pool(name="stat", bufs=1))
    # PSUM budget (8 banks): psA bufs=2 {P:[8,512]x2=2, rA:[8,264]x2=2},
    # psB bufs=1 {rB:[8,264]=1, oh:[8,1024]=2, T:[128,176]=1} -> 8 banks.
    psA = ctx.enter_context(tc.tile_pool(name="psA", bufs=2, space="PSUM"))
    psB = ctx.enter_context(tc.tile_pool(name="psB", bufs=1, space="PSUM"))

    # ---- constants ----
    ident = consts.tile([BB, BB], BF16, tag="ident")
    nc.sync.dma_start(ident[:, :], io["ident"][:, :])
    dpar = consts.tile([BB, 8 * LAYERS], F32, tag="dpar")
    nc.sync.dma_start(dpar[:, :], io["dparam"][:, :])
    mapm = consts.tile([BB, OHM], F32, tag="mapm")
    nc.sync.dma_start(mapm[:, :], io["mapp_m"][:, :])
    mapp = consts.tile([BB, OHP], F32, tag="mapp")
    nc.sync.dma_start(mapp[:, :], io["mapp_p"][:, :])
    pi2 = consts.tile([BB, 1], F32, tag="pi2")
    nc.vector.memset(pi2[:, :], PI2)

    # persistent zero-masked stationary tiles
    # xstat: fwd-matvec stationaries: region (t,b) at cols [64t+8b, +8), col b hot
    xstat = stat.tile([128, 4 * 64], BF16, tag="xstat")
    nc.vector.memset(xstat[:, :], 0.0)
    # xrem: remainder stationaries, 32-aligned: half h holds b=4h..4h+3 at
    # partitions 32(b%4)..+16, column b; other columns zero.
    xrem = []
    for h in range(2):
        xr = stat.tile([128, 8], BF16, tag=f"xrem{h}", name=f"xrem{h}")
        nc.vector.memset(xr[:, :], 0.0)
        xrem.append(xr)
    # gstat: r-matvec stationaries: region (T,t,b) at cols [256T+64t+8b, +8)
    gstat = stat.tile([128, 2 * 4 * 64], BF16, tag="gstat")
    nc.vector.memset(gstat[:, :], 0.0)

    NJT = _ceil_div(JD, 128)       # 5 j-tiles (4 full + 16-row remainder)
    NKT = KD // 128                # 4 k-tiles

    for blk in range(NBLK):
        b0 = blk * BB
        # ---- load per-block data ----
        y_e = work1.tile([BB, BL], F32, tag="y_e")
        nc.sync.dma_start(y_e[:, :], io["y_e"][b0:b0 + BB, :])
        y_o = work1.tile([BB, BL], F32, tag="y_o")
        nc.sync.dma_start(y_o[:, :], io["y_o"][b0:b0 + BB, :])

        nat = {}
        ptr = {}
        for S, name in ((0, "e"), (1, "o")):
            for b in range(BB):
                t_n = psi.tile([128, 4 * JD], BF16, tag=f"nat_{name}{b}")
                nc.sync.dma_start(t_n[:, :], io[f"pn_{name}"][b0 + b, :, :])
                nat[(S, b)] = t_n
                t_t = psi.tile([128, 4 * 512], BF16, tag=f"pt_{name}{b}")
                nc.sync.dma_start(t_t[:, :], io[f"pt_{name}"][b0 + b, :, :])
                ptr[(S, b)] = t_t
        trem = {}
        for S, name in ((0, "e"), (1, "o")):
            pair = []
            for h in range(2):
                t_r = psi.tile([128, 512], BF16, tag=f"trem_{name}{h}",
                               name=f"trem_{name}{h}")
                nc.sync.dma_start(t_r[:, :], io[f"trem_{name}"][blk, h, :, :])
                pair.append(t_r)
            trem[S] = pair

        # ---- init carries (share the rotating stage_new tags) ----
        mag = work.tile([BB, SL], F32, tag="new0", name="mag0")
        nc.vector.memset(mag[:, :], 0.0)
        ph = work.tile([BB, SL], F32, tag="new1", name="ph0")
        nc.vector.memset(ph[:, :], 0.0)
        xoh_m = carry.tile([BB, SL * OHM], F32, tag="xoh_m")
        nc.vector.memset(xoh_m[:, :], 0.0)
        xoh_p = carry.tile([BB, SL * OHP], F32, tag="xoh_p")
        nc.vector.memset(xoh_p[:, :], 0.0)
        v_m = carry.tile([BB, VL], F32, tag="v_m")
        nc.vector.memset(v_m[:, :], 0.0)
        v_p = carry.tile([BB, VL], F32, tag="v_p")
        nc.vector.memset(v_p[:, :], 0.0)

        for l in range(LAYERS):
            # ---- stream this layer's weights (prepacked SBUF images) ----
            w1 = {}
            w3 = {}
            for s in range(2):
                w1[s] = wpool.tile([128, 4 * 512], BF16, tag="w1",
                                   name=f"w1_{s}")
                nc.sync.dma_start(w1[s][:, :], io["w1"][l, s, :, :])
                w3[s] = wpool.tile([128, 512 + 128], BF16, tag="w3",
                                   name=f"w3_{s}")
                nc.sync.dma_start(w3[s][:, :], io["w3"][l, s, :, :])
            w2chunks = {0: [], 1: []}
            for s, nch in ((0, 1), (1, 2)):
                for c in range(nch):
                    w2c = wpool.tile([128, 4 * 1056 + 1056], BF16, tag="w2")
                    src = io["w2m"] if s == 0 else io["w2p"]
                    nc.sync.dma_start(w2c[:, :], src[l * nch + c, :, :])
                    w2chunks[s].append(w2c)

            # ---- per-layer trig ----
            cos = work1.tile([BB, SL], F32, tag="cos")
            nc.scalar.activation(cos[:, :], ph[:, :], AF.Sin, bias=pi2[:, :])
            sin = work1.tile([BB, SL], F32, tag="sin")
            nc.scalar.activation(sin[:, :], ph[:, :], AF.Sin)

            stage_new = {}
            for s in range(2):  # 0 = mag, 1 = phase
                dcol = l * 8 + s * 4
                oh = OHM if s == 0 else OHP
                ohw = SL * oh
                xoh = xoh_m if s == 0 else xoh_p
                vcar = v_m if s == 0 else v_p
                xprev = mag if s == 0 else ph
                mapt = mapm if s == 0 else mapp

                # -d1*y_e and -d2*y_o  (per stage: d from this stage's params)
                nd1y = work1.tile([BB, BL], F32, tag="nd1y")
                nc.vector.tensor_scalar_mul(nd1y[:, :], y_e[:, :],
                                            dpar[:, dcol + 0:dcol + 1])
                nd2y = work1.tile([BB, BL], F32, tag="nd2y")
                nc.vector.tensor_scalar_mul(nd2y[:, :], y_o[:, :],
                                            dpar[:, dcol + 2:dcol + 3])

                # ---- xvec (bf16) and c,s tensors ----
                xv = work1.tile([BB, JD], BF16, tag="xv")
                if s == 0:
                    c_t, s_t, c_neg = cos, sin, False
                    nc.vector.tensor_tensor(xv[:, 0:SL], mag[:, :], cos[:, :],
                                            ALU.mult)
                    nc.vector.tensor_tensor(xv[:, SL:JD], mag[:, :], sin[:, :],
                                            ALU.mult)
                else:
                    magn = stage_new[0]
                    tcf = work1.tile([BB, SL], F32, tag="sA", name="tcf")
                    nc.vector.tensor_tensor(tcf[:, :], sin[:, :], magn[:, :],
                                            ALU.mult)
                    tsf = work1.tile([BB, SL], F32, tag="sB", name="tsf")
                    nc.vector.tensor_tensor(tsf[:, :], cos[:, :], magn[:, :],
                                            ALU.mult)
                    # xvec2 = [magn*cos | magn*sin] = [tsf | tcf]
                    nc.vector.tensor_copy(xv[:, 0:SL], tsf[:, :])
                    nc.vector.tensor_copy(xv[:, SL:JD], tcf[:, :])
                    c_t, s_t, c_neg = tcf, tsf, True  # c = -sin*magn

                # ---- transpose xvec -> xstat / xrem ----
                psT = psB.tile([128, 176], BF16, tag="T")
                for t in range(4):
                    nc.tensor.transpose(psT[:, 8 * t:8 * t + 8],
                                        xv[:, 128 * t:128 * (t + 1)],
                                        ident[0:BB, 0:BB])
                nc.tensor.transpose(psT[0:16, 32:40], xv[:, 512:JD],
                                    ident[0:BB, 0:BB])
                # hot-col scatter: psT[p, 8t+b] -> xstat[p, 64t+9b]
                nc.vector.tensor_copy(
                    _strided(xstat[:, :], [[64, 4], [9, 8]]),
                    psT[:, 0:32].rearrange("p (t b) -> p t b", t=4),
                )
                for b in range(BB):
                    h, bq = b // 4, b % 4
                    nc.vector.tensor_copy(
                        xrem[h][32 * bq:32 * bq + 16, b:b + 1],
                        psT[0:16, 32 + b:33 + b])

                # ---- forward matvecs: P = Psi @ xvec  -> [8, 512] psum ----
                psP = {}
                for S in range(2):
                    pp = psA.tile([BB, KD], F32, tag="P")
                    n_mm = BB * 4 + 1
                    i = 0
                    for b in range(BB):
                        for t in range(4):
                            nc.tensor.matmul(
                                pp[:, :], xstat[:, 64 * t + 8 * b:64 * t + 8 * b + 8],
                                ptr[(S, b)][:, 512 * t:512 * (t + 1)],
                                start=(i == 0), stop=False)
                            i += 1
                    for h in range(2):
                        nc.tensor.matmul(pp[:, :], xrem[h][:, :],
                                         trem[S][h][:, :],
                                         start=False, stop=(h == 1))
                    psP[S] = pp

                # ---- we/wo, u, gh ----
                gh = {}
                for S in range(2):
                    dw = dcol + (1 if S == 0 else 3)
                    ndyt = nd1y if S == 0 else nd2y
                    sq = work1.tile([BB, KD], F32, tag="sq")
                    nc.scalar.square(sq[:, :], psP[S][:, :])
                    we = work1.tile([BB, BL], F32, tag="we")
                    nc.vector.tensor_tensor(we[:, :], sq[:, 0:BL], sq[:, BL:KD],
                                            ALU.add)
                    u = work1.tile([BB, BL], F32, tag="u")
                    nc.vector.scalar_tensor_tensor(
                        u[:, :], we[:, :], dpar[:, dw:dw + 1], ndyt[:, :],
                        op0=ALU.mult, op1=ALU.add)
                    g = work.tile([BB, KD], BF16, tag="gh")
                    urep = u[:, :].unsqueeze(1).broadcast_to((BB, 2, BL))
                    nc.vector.tensor_tensor(
                        g[:, :].rearrange("p (r k) -> p r k", r=2),
                        psP[S][:, :].rearrange("p (r k) -> p r k", r=2),
                        urep, ALU.mult)
                    gh[S] = g

                # ---- transpose gh -> gstat (per tensor, so r_e starts early)
                psT2 = psB.tile([128, 176], BF16, tag="T")
                for S in range(2):
                    for t in range(4):
                        nc.tensor.transpose(
                            psT2[:, 32 * S + 8 * t:32 * S + 8 * t + 8],
                            gh[S][:, 128 * t:128 * (t + 1)], ident[0:BB, 0:BB])
                    gsl = _AP(gstat[:, :].tensor, gstat[:, :].offset + 256 * S,
                              [list(gstat[:, :].ap[0]), [64, 4], [9, 8]])
                    nc.vector.tensor_copy(
                        gsl,
                        psT2[:, 32 * S:32 * S + 32].rearrange(
                            "p (t b) -> p t b", t=4))

                # ---- r matvecs: r = Psi_e.T@gh_e + Psi_o.T@gh_o ----
                rA = psA.tile([BB, SL], F32, tag="rA")
                rB = psB.tile([BB, SL], F32, tag="rB")
                n_mm = 2 * 4 * BB
                i = 0
                for S in range(2):
                    for t in range(4):
                        for b in range(BB):
                            lhs = gstat[:, 256 * S + 64 * t + 8 * b:
                                        256 * S + 64 * t + 8 * b + 8]
                            nc.tensor.matmul(
                                rA[:, :], lhs,
                                nat[(S, b)][:, JD * t:JD * t + SL],
                                start=(i == 0), stop=(i == n_mm - 1))
                            nc.tensor.matmul(
                                rB[:, :], lhs,
                                nat[(S, b)][:, JD * t + SL:JD * (t + 1)],
                                start=(i == 0), stop=(i == n_mm - 1))
                            i += 1

                # ---- q = xprev + c*rA + s*rB ; act = [q | v] bf16 ----
                act = work1.tile([BB, 400], BF16, tag="act")
                t1 = work1.tile([BB, SL], F32, tag="sC", name="t1")
                if c_neg:
                    nc.vector.scalar_tensor_tensor(
                        t1[:, :], rA[:, :], -1.0, c_t[:, :],
                        op0=ALU.mult, op1=ALU.mult)
                else:
                    nc.vector.tensor_tensor(t1[:, :], rA[:, :], c_t[:, :],
                                            ALU.mult)
                t2 = work1.tile([BB, SL], F32, tag="t2")
                nc.vector.tensor_tensor(t2[:, :], rB[:, :], s_t[:, :], ALU.mult)
                nc.vector.tensor_tensor(t1[:, :], t1[:, :], t2[:, :], ALU.add)
                nc.vector.tensor_tensor(act[:, 0:SL], t1[:, :], xprev[:, :],
                                        ALU.add)
                nc.vector.tensor_copy(act[:, SL:SL + VL], vcar[:, :])
                nc.vector.memset(act[:, SL + VL:SL + VL + 1], 1.0)

                # ---- actT (with ones row for W1 bias) ----
                psT3 = psB.tile([128, 176], BF16, tag="T")
                for t in range(3):
                    nc.tensor.transpose(psT3[:, 40 + 8 * t:48 + 8 * t],
                                        act[:, 128 * t:128 * (t + 1)],
                                        ident[0:BB, 0:BB])
                nc.tensor.transpose(psT3[0:16, 64:72], act[:, 384:400],
                                    ident[0:BB, 0:BB])
                actT = work1.tile([128, 32], BF16, tag="actT")
                nc.scalar.copy(actT[:, :], psT3[:, 40:72])

                # ---- z = clip(act @ W1.T + b1) ----
                psz = psA.tile([BB, ZL], F32, tag="P")
                for t in range(3):
                    nc.tensor.matmul(psz[:, :], actT[:, 8 * t:8 * t + 8],
                                     w1[s][:, 512 * t:512 * (t + 1)],
                                     start=(t == 0), stop=False)
                nc.tensor.matmul(psz[:, :], actT[0:9, 24:32],
                                 w1[s][0:9, 1536:2048], start=False, stop=True)
                z = work1.tile([BB, ZL], BF16, tag="z")
                nc.vector.tensor_scalar(z[:, :], psz[:, :], -10.0, 10.0,
                                        op0=ALU.max, op1=ALU.min)

                # ---- zT ----
                psT4 = psB.tile([128, 176], BF16, tag="T")
                for t in range(4):
                    nc.tensor.transpose(psT4[:, 72 + 8 * t:80 + 8 * t],
                                        z[:, 128 * t:128 * (t + 1)],
                                        ident[0:BB, 0:BB])
                zT = work1.tile([128, 32], BF16, tag="zT")
                nc.scalar.copy(zT[:, :], psT4[:, 72:104])

                # ---- x_oh += z @ W2.T + b2 (chunked passes) ----
                nch = 1 if s == 0 else 2
                for c in range(nch):
                    w2c = w2chunks[s][c]
                    # pass covers oh cols [1056c, 1056c+1056)
                    for part, pw in ((0, 1024), (1024, 32)):
                        if pw == 1024:
                            pso = psB.tile([BB, 1024], F32, tag="oh")
                        else:
                            pso = psA.tile([BB, SL], F32, tag="rA")
                        for nc0 in range(0, pw, 512):
                            nw = min(512, pw - nc0)
                            off = part + nc0
                            for t in range(4):
                                nc.tensor.matmul(
                                    pso[:, nc0:nc0 + nw],
                                    zT[:, 8 * t:8 * t + 8],
                                    w2c[:, 1056 * t + off:1056 * t + off + nw],
                                    start=(t == 0), stop=False)
                            nc.tensor.matmul(
                                pso[:, nc0:nc0 + nw], ident[0:BB, 0:BB],
                                w2c[0:BB, 4224 + off:4224 + off + nw],
                                start=False, stop=True)
                        base = 1056 * c + part
                        nc.vector.tensor_tensor(
                            xoh[:, base:base + pw], pso[:, 0:pw],
                            xoh[:, base:base + pw], ALU.add)

                # ---- v += z @ W3.T + b3 ----
                psv = psB.tile([BB, VL], F32, tag="rB")
                for t in range(4):
                    nc.tensor.matmul(psv[:, :], zT[:, 8 * t:8 * t + 8],
                                     w3[s][:, 128 * t:128 * (t + 1)],
                                     start=(t == 0), stop=False)
                nc.tensor.matmul(psv[:, :], ident[0:BB, 0:BB],
                                 w3[s][0:BB, 512:640], start=False, stop=True)
                nc.vector.tensor_tensor(vcar[:, :], psv[:, :], vcar[:, :],
                                        ALU.add)

                # ---- oh2sym: softmax-weighted sum (chunked) ----
                nsm = ohw // 528           # mag: 2 passes, ph: 4 passes
                sw = SL // nsm
                den = work1.tile([BB, SL], F32, tag="sA", name="den")
                numr = work1.tile([BB, SL], F32, tag="sB", name="numr")
                for csm in range(nsm):
                    esb = work1.tile([BB, 528], F32, tag="esb", name="esb")
                    nc.scalar.activation(esb[:, :],
                                         xoh[:, 528 * csm:528 * (csm + 1)],
                                         AF.Exp)
                    e3 = esb[:, :].rearrange("p (s k) -> p s k", k=oh)
                    nc.vector.tensor_reduce(den[:, sw * csm:sw * (csm + 1)],
                                            e3, AX.X, ALU.add)
                    maprep = mapt[:, :].unsqueeze(1).broadcast_to((BB, sw, oh))
                    nc.gpsimd.tensor_tensor(e3, e3, maprep, ALU.mult)
                    nc.vector.tensor_reduce(numr[:, sw * csm:sw * (csm + 1)],
                                            e3, AX.X, ALU.add)
                rden = work1.tile([BB, SL], F32, tag="sC", name="rden")
                nc.vector.reciprocal(rden[:, :], den[:, :])
                newsym = work.tile([BB, SL], F32, tag=f"new{s}")
                nc.vector.tensor_tensor(newsym[:, :], numr[:, :], rden[:, :],
                                        ALU.mult)
                stage_new[s] = newsym
                dst = io["mag_out"] if s == 0 else io["ph_out"]
                nc.sync.dma_start(dst[l, b0:b0 + BB, :], newsym[:, :])

            # rotate carries for next layer
            mag = stage_new[0]
            ph = stage_new[1]

    ctx.close()


# ---------------------------------------------------------------------------
# host-side packing
# ---------------------------------------------------------------------------

def _pack_psi_nat(psi):
    # (B, 512, 528) f32 -> (B, 128, 4*528) bf16 image [k-part, j-free]
    B = psi.shape[0]
    return np.ascontiguousarray(
        psi.reshape(B, 4, 128, JD).transpose(0, 2, 1, 3).reshape(B, 128, 4 * JD)
    ).astype(bfloat16)


def _pack_psi_T(psi):
    # transposed layout images: main (B, 128, 4*512) + remainder per block
    B = psi.shape[0]
    pT = psi.transpose(0, 2, 1)  # (B, 528, 512)
    main = np.ascontiguousarray(
        pT[:, 0:512, :].reshape(B, 4, 128, 512).transpose(0, 2, 1, 3)
        .reshape(B, 128, 4 * 512)).astype(bfloat16)
    rem = np.ascontiguousarray(pT[:, 512:JD, :]).astype(bfloat16)  # (B,16,512)
    return main, rem


def _pack_trem(rem, nblk, bb):
    # (Bcore, 16, 512) -> (nblk, 2, 128, 512), 32-aligned block-diagonal
    out = np.zeros((nblk, 2, 128, 512), dtype=bfloat16)
    for k in range(nblk):
        for b in range(bb):
            h, bq = b // 4, b % 4
            out[k, h, 32 * bq:32 * bq + 16, :] = rem[k * bb + b]
    return out


def _pack_w1(W1, b1):
    # W1 (L, 512, 392), b1 (L, 512) -> (L, 128, 2048) image of W1.T+bias
    L = W1.shape[0]
    w1t = np.concatenate([W1.transpose(0, 2, 1), b1[:, None, :]], axis=1)
    out = np.zeros((L, 128, 4 * 512), dtype=bfloat16)
    for t in range(3):
        out[:, :, 512 * t:512 * (t + 1)] = w1t[:, 128 * t:128 * (t + 1), :]
    out[:, 0:W1K - 384, 1536:2048] = w1t[:, 384:W1K, :]
    return out


def _pack_w2(W2, b2, bb):
    # W2 (L, ohw, 512), b2 (L, ohw) -> (L*nch, 128, 5280) chunk images
    L, ohw, _ = W2.shape
    nch = ohw // 1056
    w2t = W2.transpose(0, 2, 1)  # (L, 512, ohw)
    out = np.zeros((L * nch, 128, 4 * 1056 + 1056), dtype=bfloat16)
    for l in range(L):
        for c in range(nch):
            cols = slice(1056 * c, 1056 * (c + 1))
            for t in range(4):
                out[l * nch + c, :, 1056 * t:1056 * (t + 1)] = \
                    w2t[l, 128 * t:128 * (t + 1), cols]
            out[l * nch + c, 0:bb, 4224:5280] = \
                np.broadcast_to(b2[l, cols], (bb, 1056))
    return out


def _pack_w3(W3, b3, bb):
    # W3 (L, 128, 512), b3 (L, 128) -> (L, 128, 640)
    L = W3.shape[0]
    w3t = W3.transpose(0, 2, 1)  # (L, 512, 128)
    out = np.zeros((L, 128, 512 + 128), dtype=bfloat16)
    for t in range(4):
        out[:, :, 128 * t:128 * (t + 1)] = w3t[:, 128 * t:128 * (t + 1), :]
    out[:, 0:bb, 512:640] = np.broadcast_to(b3[:, None, :], (L, bb, 128))
    return out


_CACHE = {}
LAST_EXEC_NS = None


def _build_program():
    if "nc" in _CACHE:
        return _CACHE["nc"]
    nc = bacc.Bacc("TRN2", target_bir_lowering=False, debug=False,
                   num_devices=NCORES)
    io = {}

    def inp(name, shape, dt):
        io[name] = nc.dram_tensor(name, list(shape), dt,
                                  kind="ExternalInput").ap()

    inp("pn_e", (BCORE, 128, 4 * JD), BF16)
    inp("pn_o", (BCORE, 128, 4 * JD), BF16)
    inp("pt_e", (BCORE, 128, 4 * 512), BF16)
    inp("pt_o", (BCORE, 128, 4 * 512), BF16)
    inp("trem_e", (NBLK, 2, 128, 512), BF16)
    inp("trem_o", (NBLK, 2, 128, 512), BF16)
    inp("y_e", (BCORE, BL), F32)
    inp("y_o", (BCORE, BL), F32)
    inp("w1", (LAYERS, 2, 128, 4 * 512), BF16)
    inp("w2m", (LAYERS * 1, 128, 5280), BF16)
    inp("w2p", (LAYERS * 2, 128, 5280), BF16)
    inp("w3", (LAYERS, 2, 128, 640), BF16)
    inp("dparam", (BB, 8 * LAYERS), F32)
    inp("mapp_m", (BB, OHM), F32)
    inp("mapp_p", (BB, OHP), F32)
    inp("ident", (BB, BB), BF16)
    io["mag_out"] = nc.dram_tensor("mag_out", [LAYERS, BCORE, SL], F32,
                                   kind="ExternalOutput").ap()
    io["ph_out"] = nc.dram_tensor("ph_out", [LAYERS, BCORE, SL], F32,
                                  kind="ExternalOutput").ap()

    with tile.TileContext(nc) as tc:
        emit_core_program(nc, tc, io)
    nc.compile()
    _CACHE["nc"] = nc
    return nc


def kernel(y_e, y_o, Psi_e, Psi_o, mapp_mag, mapp_phase, params):
    y_e = np.asarray(y_e, np.float32)
    y_o = np.asarray(y_o, np.float32)
    Psi_e = np.asarray(Psi_e, np.float32)
    Psi_o = np.asarray(Psi_o, np.float32)
    P = {k: np.asarray(v, np.float32) for k, v in params.items()}

    nc = _build_program()

    # shared (batch-independent) arrays
    w1_m = _pack_w1(P["W1_mag"][:LAYERS], P["b1_mag"][:LAYERS])
    w1_p = _pack_w1(P["W1_phase"][:LAYERS], P["b1_phase"][:LAYERS])
    w1 = np.stack([w1_m, w1_p], axis=1)
    w2m = _pack_w2(P["W2_mag"][:LAYERS], P["b2_mag"][:LAYERS], BB)
    w2p = _pack_w2(P["W2_phase"][:LAYERS], P["b2_phase"][:LAYERS], BB)
    w3_m = _pack_w3(P["W3_mag"][:LAYERS], P["b3_mag"][:LAYERS], BB)
    w3_p = _pack_w3(P["W3_phase"][:LAYERS], P["b3_phase"][:LAYERS], BB)
    w3 = np.stack([w3_m, w3_p], axis=1)
    dparam = np.zeros((BB, 8 * LAYERS), np.float32)
    for l in range(LAYERS):
        for s, tag in enumerate(["mag", "phase"]):
            d1 = P["d1_" + tag][l, 0]
            d2 = P["d2_" + tag][l, 0]
            d4 = P["d4_" + tag][l, 0]
            dparam[:, l * 8 + s * 4 + 0] = -d1
            dparam[:, l * 8 + s * 4 + 1] = d2
            dparam[:, l * 8 + s * 4 + 2] = -d2
            dparam[:, l * 8 + s * 4 + 3] = d4
    mapp_m = np.broadcast_to(np.asarray(mapp_mag, np.float32), (BB, OHM)).copy()
    mapp_p = np.broadcast_to(np.asarray(mapp_phase, np.float32),
                             (BB, OHP)).copy()
    ident = np.eye(BB, dtype=bfloat16)

    shared = dict(w1=w1, w2m=w2m, w2p=w2p, w3=w3, dparam=dparam,
                  mapp_m=mapp_m, mapp_p=mapp_p, ident=ident)

    in_maps = []
    for c in range(NCORES):
        sl_ = slice(c * BCORE, (c + 1) * BCORE)
        pn_e = _pack_psi_nat(Psi_e[sl_])
        pn_o = _pack_psi_nat(Psi_o[sl_])
        pt_e, rem_e = _pack_psi_T(Psi_e[sl_])
        pt_o, rem_o = _pack_psi_T(Psi_o[sl_])
        m = dict(shared)
        m.update(pn_e=pn_e, pn_o=pn_o, pt_e=pt_e, pt_o=pt_o,
                 trem_e=_pack_trem(rem_e, NBLK, BB),
                 trem_o=_pack_trem(rem_o, NBLK, BB),
                 y_e=np.ascontiguousarray(y_e[sl_]),
                 y_o=np.ascontiguousarray(y_o[sl_]))
        in_maps.append(m)

    trace = bool(int(os.environ.get("DETNET_TRACE", "0")))
    res = run_bass_kernel_spmd(nc, in_maps, list(range(NCORES)), trace=trace)
    global LAST_EXEC_NS
    LAST_EXEC_NS = res.exec_time_ns
    mags = np.concatenate([res.results[c]["mag_out"] for c in range(NCORES)],
                          axis=1)
    phs = np.concatenate([res.results[c]["ph_out"] for c in range(NCORES)],
                         axis=1)
    return mags, phs


if __name__ == "__main__":
    print("kernel module loaded; run test.py")
